# revision 24
# baseline (speedup 1.0000x reference)
"""Trainium2 Bass kernel for spatial attention (GroupNorm + QKV + softmax attention
+ output projection + residual), distributed over 8 NeuronCores.

Sharding: core = 2*b + hp handles image b (of 4) and head pair hp (heads 2hp, 2hp+1).
Each core computes GroupNorm(x[b]), its heads' q/k/v, full spatial attention for its
two heads, and a partial output projection (out_w columns for its heads). Core pairs'
partial outputs plus the residual x + out_b are summed on the host.

Perf structure:
- softmax exp() is the serial bottleneck (10.6M elements/core): split across
  ScalarE (exact exp, 9/18 key chunks) and VectorE (Schraudolph bf16 bit-trick
  via tensor_scalar -> uint16 bitcast view, 9/18 chunks).
- attention runs as one flat (i-block, j-chunk) stream with the S matmuls
  emitted 4 steps ahead of PV so the in-order PE queue never interleaves a
  PV (which waits on exp) in front of an S (which feeds the next exp):
  st is triple-buffered in PSUM, u single-buffered (8 banks total).
- softmax normalize: ScalarE stages u to SBUF, VectorE takes the reciprocal,
  GpSimd broadcasts and multiplies into headout (GpSimd cannot touch PSUM).
- x DMA first; res input dropped (host adds x + out_b).
"""

import numpy as np

import concourse.bass as bass
import concourse.bacc as bacc
import concourse.tile as tile
from concourse import mybir
from concourse import bass_utils
from concourse.alu_op_type import AluOpType

B, C, H, W = 4, 256, 48, 48
HW = H * W  # 2304
NH, HD = 4, 64
G, GC = 16, 16  # 16 groups x 16 channels
EPS = 1e-5
NCORES = 8
JC = 128  # j (key spatial) chunk
NJ = HW // JC  # 18
IBLKS = [(0, 512), (512, 1024), (1024, 1536), (1536, 2048), (2048, 2304)]
NB = len(IBLKS)
HALF = HW // 2  # 1152
SALIGN = 512
LOOKAHEAD = 4

F32 = mybir.dt.float32
BF16 = mybir.dt.bfloat16
U16 = mybir.dt.uint16
AX = mybir.AxisListType.X
AF = mybir.ActivationFunctionType
OP = AluOpType

# Schraudolph exp in bf16-bit space: exp(s/16) ~= bitcast_bf16(rint(s*A + B)).
# A = 128*log2(e)/16; B = 127*128 - C_mean + 0.25, C_mean centers the mean
# multiplicative error of the (1+f) mantissa approximation (~= 7.364).
A_SCH = 11.541560327111707
B_SCH = 16256.0 - 7.364191473886416 + 0.25
# j-chunks whose exp runs on VectorE (rest on ScalarE)
DVE_JC = frozenset((1, 3, 5, 7, 9, 11, 13, 15))


def _nchunks(size, step=512):
    # PSUM-bank-aligned chunks: a matmul output may not cross a 512-fp32 bank boundary
    return [(a, min(a + step, size)) for a in range(0, size, step)]


def _build(mm_dt=BF16):
    nc = bacc.Bacc("TRN2", target_bir_lowering=False, debug=False, enable_asserts=False)

    x_d = nc.dram_tensor("x", [C, HW], F32, kind="ExternalInput").ap()
    # wq|wk|wv packed [C, 3*128]; wo [128, C]; gnp|gind packed [128, 34]
    wqkv_d = nc.dram_tensor("wqkv", [C, 6 * HD], F32, kind="ExternalInput").ap()
    wo_d = nc.dram_tensor("wo", [2 * HD, C], F32, kind="ExternalInput").ap()
    gg_d = nc.dram_tensor("gg", [128, 36], F32, kind="ExternalInput").ap()
    gbc_d = nc.dram_tensor("gbc", [16, C], F32, kind="ExternalInput").ap()
    y_d = nc.dram_tensor("y", [C, HW], mybir.dt.float16, kind="ExternalOutput").ap()

    with tile.TileContext(nc) as tc:
        with (
            tc.tile_pool(name="consts", bufs=1) as consts,
            tc.tile_pool(name="big", bufs=1) as big,
            tc.tile_pool(name="small", bufs=4) as small,
            tc.tile_pool(name="pt", bufs=8) as ptp,
        ):
            # ---- x first: it gates GroupNorm; spread enqueues across engines ----
            x_sb = [big.tile([128, HW], F32, tag=f"x{ct}", name=f"x{ct}") for ct in range(2)]
            # x0 first; x1 queued behind the small weight DMAs so the DMA engines
            # finish x0 early and tile-0 GroupNorm can start ~6us in
            nc.sync.dma_start(x_sb[0][:], x_d[0:128, :])
            gg_sb = consts.tile([128, 36], F32, tag="gg", name="gg")
            nc.gpsimd.dma_start(gg_sb[:], gg_d[:])
            gnp_sb = [gg_sb[:, 0:2], gg_sb[:, 2:4]]  # per-tile (weight, bias)
            gind_sb = gg_sb[:, 4:36]
            gbc_sb = consts.tile([16, C], F32, tag="gbc", name="gbc")
            nc.gpsimd.dma_start(gbc_sb[:], gbc_d[:])

            # preload the Exp ACT table set during the x DMA (the only set used)
            warm = small.tile([1, 2], F32, tag="warm", name="warm")
            nc.scalar.activation(warm[:], gg_sb[0:1, 0:2], AF.Exp, scale=0.0)

            # ---- weights (one DMA; casts on GpSimd to keep DVE free for GN) ----
            wqkvf = consts.tile([128, 2 * 6 * HD], F32, tag="wqkvf", name="wqkvf")
            wqf3 = wqkvf[:].rearrange("p (kc w) -> p kc w", kc=2)
            nc.sync.dma_start(
                wqf3, wqkv_d[:].rearrange("(kc p) w -> p kc w", kc=2)
            )
            w_sb = {}
            for wi, name in enumerate(("wq", "wk", "wv")):
                for kc in range(2):
                    w_sb[name, kc] = consts.tile(
                        [128, 2 * HD], mm_dt, tag=f"{name}{kc}", name=f"{name}{kc}"
                    )

            def emit_weight_casts(names):
                for wi, name in enumerate(("wq", "wk", "wv")):
                    if name not in names:
                        continue
                    for kc in range(2):
                        nc.vector.tensor_copy(
                            w_sb[name, kc][:], wqf3[:, kc, wi * 128 : (wi + 1) * 128]
                        )
            wof = consts.tile([128, C], F32, tag="wof", name="wof")
            nc.sync.dma_start(wof[:], wo_d[:])
            nc.scalar.dma_start(x_sb[1][:], x_d[128:256, :])
            wo_sb = consts.tile([128, C], mm_dt, tag="wo", name="wo")

            xn_sb = [big.tile([128, HW], mm_dt, tag=f"xn{ct}", name=f"xn{ct}") for ct in range(2)]

            # ---- GroupNorm ----
            stats = small.tile([128, 4], F32, tag="stats", name="stats")
            for ct in range(2):
                nc.vector.reduce_sum(stats[:, 2 * ct : 2 * ct + 1], x_sb[ct][:], axis=AX)
                nc.vector.scalar_tensor_tensor(
                    xn_sb[ct][:], x_sb[ct][:], 1.0, x_sb[ct][:],
                    op0=OP.mult, op1=OP.mult,
                    accum_out=stats[:, 2 * ct + 1 : 2 * ct + 2],
                )
            with tc.tile_pool(name="ps_gn", bufs=2, space=bass.MemorySpace.PSUM) as ps_gn:
                g_ps = ps_gn.tile([16, 2], F32, tag="g", name="g")
                nc.tensor.matmul(g_ps[:], gind_sb[:, 0:16], stats[:, 0:2], start=True, stop=False)
                nc.tensor.matmul(g_ps[:], gind_sb[:, 16:32], stats[:, 2:4], start=False, stop=True)
                mall = small.tile([16, 2], F32, tag="mall", name="mall")
                nc.vector.tensor_scalar_mul(mall[:], g_ps[:], 1.0 / (GC * HW))
                msq = small.tile([16, 1], F32, tag="msq", name="msq")
                nc.vector.tensor_tensor(msq[:], mall[:, 0:1], mall[:, 0:1], op=OP.mult)
                ve = small.tile([16, 1], F32, tag="ve", name="ve")
                nc.vector.tensor_tensor(ve[:], mall[:, 1:2], msq[:], op=OP.subtract)
                ve2 = small.tile([16, 1], F32, tag="ve2", name="ve2")
                nc.vector.tensor_scalar_add(ve2[:], ve[:], EPS)
                # rstd via Quake rsqrt bit-trick + 2 Newton steps, all on DVE
                sh = small.tile([16, 1], mybir.dt.uint32, tag="sh", name="sh")
                nc.vector.tensor_scalar(
                    sh[:], ve2[:].bitcast(mybir.dt.uint32), 1, None,
                    op0=OP.logical_shift_right,
                )
                nt = small.tile([16, 1], mybir.dt.uint32, tag="nt", name="nt")
                nc.vector.tensor_scalar(
                    nt[:], sh[:], 0xFFFFFFFF, None, op0=OP.bitwise_xor
                )
                r0u = small.tile([16, 1], mybir.dt.uint32, tag="r0u", name="r0u")
                nc.vector.tensor_scalar(
                    r0u[:], nt[:], 0xFFFFFFFF - 0x5F3759DF, None, op0=OP.subtract
                )
                r = r0u[:].bitcast(F32)
                gvals = small.tile([16, 2], F32, tag="gvals", name="gvals")
                nc.vector.tensor_copy(gvals[:, 0:1], mall[:, 0:1])
                for it in range(2):
                    rr = small.tile([16, 1], F32, tag=f"rr{it}", name=f"rr{it}")
                    nc.vector.tensor_tensor(rr[:], r, r, op=OP.mult)
                    hv = small.tile([16, 1], F32, tag=f"hv{it}", name=f"hv{it}")
                    nc.vector.tensor_tensor(hv[:], ve2[:], rr[:], op=OP.mult)
                    h2 = small.tile([16, 1], F32, tag=f"h2{it}", name=f"h2{it}")
                    nc.vector.tensor_scalar(
                        h2[:], hv[:], -0.5, 1.5, op0=OP.mult, op1=OP.add
                    )
                    if it == 0:
                        rn = small.tile([16, 1], F32, tag="rn", name="rn")
                        nc.vector.tensor_tensor(rn[:], r, h2[:], op=OP.mult)
                        r = rn[:]
                    else:
                        nc.vector.tensor_tensor(gvals[:, 1:2], r, h2[:], op=OP.mult)
                for ct in range(2):
                    cv = ps_gn.tile([128, 2], F32, tag="cv", name="cv")
                    nc.tensor.matmul(
                        cv[:], gbc_sb[:, ct * 128 : (ct + 1) * 128], gvals[:],
                        start=True, stop=True,
                    )
                    scale_t = small.tile([128, 1], F32, tag="scale", name="scale")
                    nc.vector.tensor_tensor(scale_t[:], gnp_sb[ct][:, 0:1], cv[:, 1:2], op=OP.mult)
                    tb = small.tile([128, 1], F32, tag="tb", name="tb")
                    nc.vector.tensor_tensor(tb[:], cv[:, 0:1], scale_t[:], op=OP.mult)
                    bias_t = small.tile([128, 1], F32, tag="bias", name="bias")
                    nc.vector.tensor_tensor(bias_t[:], gnp_sb[ct][:, 1:2], tb[:], op=OP.subtract)
                    nc.vector.tensor_scalar(
                        xn_sb[ct][:], x_sb[ct][:], scale_t[:], bias_t[:],
                        op0=OP.mult, op1=OP.add,
                    )
                emit_weight_casts(("wq", "wk", "wv"))
                nc.vector.tensor_copy(wo_sb[:], wof[:])

            # ---- QKV projections ----
            q_sb = big.tile([128, HW], mm_dt, tag="q", name="q")
            k_sb = big.tile([128, HW], mm_dt, tag="k", name="k")
            vt_sb = []
            for h in range(2):
                t = big.tile([128, NJ * (HD + 1)], mm_dt, tag=f"vt{h}", name=f"vt{h}")
                t3 = t[:].rearrange("p (j c) -> p j c", c=HD + 1)
                # fill the per-chunk "ones" column (denominator accumulator rows)
                nc.vector.tensor_scalar(
                    t3[:, :, HD : HD + 1], x_sb[0][:, 0:NJ], 0.0, 1.0,
                    op0=OP.mult, op1=OP.add,
                )
                vt_sb.append(t)

            with tc.tile_pool(name="ps_qkv", bufs=2, space=bass.MemorySpace.PSUM) as ps_qkv:
                # v^T first: its DVE copies then overlap the q/k matmuls, and
                # q/k (which attention needs first) are ready right at the boundary
                for half in range(2):
                    vps = ps_qkv.tile([128, HALF], F32, tag="qkv", name="qkv")
                    for j9 in range(9):
                        jc = half * 9 + j9
                        for kc in range(2):
                            nc.tensor.matmul(
                                vps[:, j9 * 128 : (j9 + 1) * 128],
                                xn_sb[kc][:, jc * JC : (jc + 1) * JC],
                                w_sb["wv", kc][:],
                                start=(kc == 0), stop=(kc == 1),
                            )
                    vps3 = vps[:].rearrange("p (j c) -> p j c", c=128)
                    for h in range(2):
                        dst3 = vt_sb[h][:].rearrange("p (j c) -> p j c", c=HD + 1)
                        nc.vector.tensor_copy(
                            dst3[:, half * 9 : (half + 1) * 9, 0:HD],
                            vps3[:, :, h * HD : (h + 1) * HD],
                        )
                for dst, wname in ((q_sb, "wq"), (k_sb, "wk")):
                    for half in range(2):
                        ps = ps_qkv.tile([128, HALF], F32, tag="qkv", name="qkv")
                        for kc in range(2):
                            for n0, n1 in _nchunks(HALF):
                                nc.tensor.matmul(
                                    ps[:, n0:n1],
                                    w_sb[wname, kc][:],
                                    xn_sb[kc][:, half * HALF + n0 : half * HALF + n1],
                                    start=(kc == 0), stop=(kc == 1),
                                )
                        # ScalarE is idle during the QKV phase; keep DVE free
                        nc.scalar.copy(dst[:, half * HALF : (half + 1) * HALF], ps[:])

            # ---- attention ----
            headout = big.tile([128, HW], mm_dt, tag="headout", name="headout")
            with tc.tile_pool(name="ps_att", bufs=1, space=bass.MemorySpace.PSUM) as ps_att:

                def emit_s(b, jc):
                    i0, i1 = IBLKS[b]
                    blk = i1 - i0
                    st = ps_att.tile([128, 2 * SALIGN], F32, tag="s", name="s", bufs=3)
                    for h in range(2):
                        lhsT = k_sb[h * HD : (h + 1) * HD, jc * JC : (jc + 1) * JC]
                        for n0, n1 in _nchunks(blk, 512):
                            nc.tensor.matmul(
                                st[:, h * SALIGN + n0 : h * SALIGN + n1],
                                lhsT,
                                q_sb[h * HD : (h + 1) * HD, i0 + n0 : i0 + n1],
                                start=True, stop=True,
                            )
                    pt = ptp.tile([128, 2 * blk], mm_dt, tag="pt", name="pt")
                    if blk == SALIGN:
                        stv, ptw = st[:], pt[:]
                    else:
                        stv = st[:].rearrange("p (h x) -> p h x", h=2)[:, :, 0:blk]
                        ptw = pt[:].rearrange("p (h x) -> p h x", h=2)
                    if jc in DVE_JC:
                        ptv = ptw.bitcast(U16)
                        # two half-width ops: the DVE pipe drain (~dur-266ns)
                        # delays the st-release semaphore, which sits on the
                        # triple-buffered st recycle path -- halving op length
                        # halves that latency at +93ns engine time
                        if blk == SALIGN:
                            nc.vector.tensor_scalar(
                                ptv[:, 0:blk], stv[:, 0:blk],
                                A_SCH, B_SCH, op0=OP.mult, op1=OP.add,
                            )
                            nc.vector.tensor_scalar(
                                ptv[:, blk : 2 * blk], stv[:, blk : 2 * blk],
                                A_SCH, B_SCH, op0=OP.mult, op1=OP.add,
                            )
                        else:
                            nc.vector.tensor_scalar(
                                ptv, stv, A_SCH, B_SCH, op0=OP.mult, op1=OP.add
                            )
                    else:
                        nc.scalar.activation(ptw, stv, AF.Exp, scale=1.0 / 16.0)
                    return pt

                def emit_pv(b, jc, pt, u):
                    blk = IBLKS[b][1] - IBLKS[b][0]
                    for h in range(2):
                        lhsT = vt_sb[h][:, jc * (HD + 1) : (jc + 1) * (HD + 1)]
                        for n0, n1 in _nchunks(blk, 512):
                            nc.tensor.matmul(
                                u[h][:, n0:n1],
                                lhsT,
                                pt[:, h * blk + n0 : h * blk + n1],
                                start=(jc == 0), stop=(jc == NJ - 1),
                            )

                def emit_norm(u, b):
                    i0, i1 = IBLKS[b]
                    blk = i1 - i0
                    for h in range(2):
                        # stage the whole u tile to SBUF with ONE ScalarE copy (its
                        # cost is free-dim-bound, partitions are free) -- u's PSUM
                        # bank is then released for the next block's PV right away.
                        # reciprocal + multiply run off the copy; GpSimd does only
                        # partition_broadcast (one ucode library -- mixing op types
                        # thrashes LOAD_LIB at ~6us each)
                        dn = small.tile([1, blk], F32, tag=f"dn{h}", name=f"dn{h}")
                        nc.scalar.copy(dn[:], u[h][HD : HD + 1, :])
                        rc = small.tile([1, blk], F32, tag=f"rc{h}", name=f"rc{h}")
                        nc.vector.reciprocal_approx_fast(rc[:], dn[:])
                        rb = small.tile([HD, blk], F32, tag=f"rb{h}", name=f"rb{h}")
                        nc.gpsimd.partition_broadcast(rb[:], rc[:])
                        nc.vector.tensor_tensor(
                            headout[h * HD : (h + 1) * HD, i0:i1],
                            u[h][0:HD, :], rb[:], op=OP.mult,
                        )

                seq = [(b, jc) for b in range(NB) for jc in range(NJ)]
                pts = {}
                emitted = 0
                state = {"u": None}
                pvq = []

                def flush_pv():
                    bb, jj, pt = pvq.pop(0)
                    if jj == 0:
                        blk = IBLKS[bb][1] - IBLKS[bb][0]
                        state["u"] = [
                            ps_att.tile([HD + 1, blk], F32, tag=f"u{h}", name=f"u{h}", bufs=1)
                            for h in range(2)
                        ]
                    emit_pv(bb, jj, pt, state["u"])

                for i, (b, jc) in enumerate(seq):
                    while emitted <= min(i + LOOKAHEAD, len(seq) - 1):
                        pts[emitted] = emit_s(*seq[emitted])
                        emitted += 1
                    pvq.append((b, jc, pts.pop(i)))
                    # lag PV emission by 2 slots: a PV blocked on exp (or, at a
                    # block start, on the previous block's normalize reading u)
                    # then sits behind 2 extra S's in the in-order PE queue, so
                    # the S->exp stream keeps both exp engines fed
                    while len(pvq) > 2:
                        flush_pv()
                    if jc == NJ - 1:
                        while pvq:
                            flush_pv()
                        emit_norm(state["u"], b)

            # ---- output projection (partial: this core's head pair) ----
            with tc.tile_pool(name="ps_out", bufs=2, space=bass.MemorySpace.PSUM) as ps_out:
                ci = 0
                for mt in range(2):
                    for n0, n1 in _nchunks(HW):
                        yp = ps_out.tile([128, 512], F32, tag="yp", name="yp")
                        nc.tensor.matmul(
                            yp[:, 0 : n1 - n0],
                            wo_sb[:, mt * 128 : (mt + 1) * 128],
                            headout[:, n0:n1],
                            start=True, stop=True,
                        )
                        yo = small.tile([128, 512], mybir.dt.float16, tag="yo", name="yo")
                        # alternate the drain copies across the two idle engines
                        if ci % 2 == 0:
                            nc.vector.tensor_copy(yo[:, 0 : n1 - n0], yp[:, 0 : n1 - n0])
                        else:
                            nc.scalar.copy(yo[:, 0 : n1 - n0], yp[:, 0 : n1 - n0])
                        ci += 1
                        nc.sync.dma_start(
                            y_d[mt * 128 : (mt + 1) * 128, n0:n1], yo[:, 0 : n1 - n0]
                        )

    nc.compile()
    return nc


def _consts():
    # gind[:, 0:16]: tile-0 channel -> group one-hot; [:, 16:32]: tile-1 channel -> group
    gind = np.zeros((128, 32), np.float32)
    for c in range(128):
        gind[c, c // GC] = 1.0
        gind[c, 16 + 8 + c // GC] = 1.0
    gbc = np.zeros((16, C), np.float32)
    for c in range(C):
        gbc[c // GC, c] = 1.0
    return gind, gbc


def make_in_maps(x, gn_weight, gn_bias, qkv_w, out_w, out_b):
    x = np.asarray(x, np.float32)
    qkv_w = np.asarray(qkv_w, np.float32)
    out_w = np.asarray(out_w, np.float32)
    gn_weight = np.asarray(gn_weight, np.float32)
    gn_bias = np.asarray(gn_bias, np.float32)
    xr = np.ascontiguousarray(x.reshape(B, C, HW))
    gind, gbc = _consts()
    # gg cols: 0:2 gnp tile0 (weight,bias), 2:4 gnp tile1, 4:36 gind one-hots
    in_maps = []
    for core in range(NCORES):
        b, hp = divmod(core, 2)
        heads = (2 * hp, 2 * hp + 1)
        qs = np.concatenate([qkv_w[n * 192 : n * 192 + 64] for n in heads], 0)
        ks = np.concatenate([qkv_w[n * 192 + 64 : n * 192 + 128] for n in heads], 0)
        vs = np.concatenate([qkv_w[n * 192 + 128 : n * 192 + 192] for n in heads], 0)
        wqkv = np.concatenate([qs.T, ks.T, vs.T], axis=1)  # [C, 384]
        gg = np.zeros((128, 36), np.float32)
        gg[:, 0] = gn_weight[0:128]
        gg[:, 1] = gn_bias[0:128]
        gg[:, 2] = gn_weight[128:256]
        gg[:, 3] = gn_bias[128:256]
        gg[:, 4:36] = gind
        in_maps.append({
            "x": xr[b],
            "wqkv": np.ascontiguousarray(wqkv),
            "wo": np.ascontiguousarray(out_w[:, hp * 128 : (hp + 1) * 128].T),
            "gg": gg,
            "gbc": gbc,
        })
    return in_maps


def assemble(results, x, out_b):
    """Sum core-pair partial outputs and add the residual + bias on the host."""
    x = np.asarray(x, np.float32)
    out_b = np.asarray(out_b, np.float32)
    xr = x.reshape(B, C, HW)
    y = np.empty((B, C, HW), np.float32)
    for b in range(B):
        y[b] = (results[2 * b]["y"].astype(np.float32)
                + results[2 * b + 1]["y"].astype(np.float32)
                + xr[b] + out_b[:, None])
    return y.reshape(B, C, H, W)


_NC_CACHE = {}


def get_nc(mm_dt=BF16):
    key = str(mm_dt)
    if key not in _NC_CACHE:
        _NC_CACHE[key] = _build(mm_dt)
    return _NC_CACHE[key]


def kernel(x, gn_weight, gn_bias, qkv_w, out_w, out_b):
    nc = get_nc(BF16)
    in_maps = make_in_maps(x, gn_weight, gn_bias, qkv_w, out_w, out_b)
    res = bass_utils.run_bass_kernel_spmd(nc, in_maps, core_ids=list(range(NCORES)))
    return assemble(res.results, x, out_b)


# revision 25
# speedup vs baseline: 1.0188x; 1.0188x over previous
"""Trainium2 Bass kernel for spatial attention (GroupNorm + QKV + softmax attention
+ output projection + residual), distributed over 8 NeuronCores.

Sharding: core = 2*b + hp handles image b (of 4) and head pair hp (heads 2hp, 2hp+1).
Each core computes GroupNorm(x[b]), its heads' q/k/v, full spatial attention for its
two heads, and a partial output projection (out_w columns for its heads). Core pairs'
partial outputs plus the residual x + out_b are summed on the host.

Perf structure:
- softmax exp() is the serial bottleneck (10.6M elements/core): split across
  ScalarE (exact exp, 9/18 key chunks) and VectorE (Schraudolph bf16 bit-trick
  via tensor_scalar -> uint16 bitcast view, 9/18 chunks).
- attention runs as one flat (i-block, j-chunk) stream with the S matmuls
  emitted 4 steps ahead of PV so the in-order PE queue never interleaves a
  PV (which waits on exp) in front of an S (which feeds the next exp):
  st is triple-buffered in PSUM, u single-buffered (8 banks total).
- softmax normalize: ScalarE stages u to SBUF, VectorE takes the reciprocal,
  GpSimd broadcasts and multiplies into headout (GpSimd cannot touch PSUM).
- x DMA first; res input dropped (host adds x + out_b).
"""

import numpy as np

import concourse.bass as bass
import concourse.bacc as bacc
import concourse.tile as tile
from concourse import mybir
from concourse import bass_utils
from concourse.alu_op_type import AluOpType

B, C, H, W = 4, 256, 48, 48
HW = H * W  # 2304
NH, HD = 4, 64
G, GC = 16, 16  # 16 groups x 16 channels
EPS = 1e-5
NCORES = 8
JC = 128  # j (key spatial) chunk
NJ = HW // JC  # 18
IBLKS = [(0, 512), (512, 1024), (1024, 1536), (1536, 2048), (2048, 2304)]
NB = len(IBLKS)
HALF = HW // 2  # 1152
SALIGN = 512
LOOKAHEAD = 4

F32 = mybir.dt.float32
BF16 = mybir.dt.bfloat16
U16 = mybir.dt.uint16
AX = mybir.AxisListType.X
AF = mybir.ActivationFunctionType
OP = AluOpType

# Schraudolph exp in bf16-bit space: exp(s/16) ~= bitcast_bf16(rint(s*A + B)).
# A = 128*log2(e)/16; B = 127*128 - C_mean + 0.25, C_mean centers the mean
# multiplicative error of the (1+f) mantissa approximation (~= 7.364).
A_SCH = 11.541560327111707
B_SCH = 16256.0 - 7.364191473886416 + 0.25
# j-chunks whose exp runs on VectorE (rest on ScalarE)
DVE_JC = frozenset((1, 3, 5, 7, 9, 11, 13, 15))


def _nchunks(size, step=512):
    # PSUM-bank-aligned chunks: a matmul output may not cross a 512-fp32 bank boundary
    return [(a, min(a + step, size)) for a in range(0, size, step)]


def _build(mm_dt=BF16):
    nc = bacc.Bacc("TRN2", target_bir_lowering=False, debug=False, enable_asserts=False)

    x_d = nc.dram_tensor("x", [C, HW], F32, kind="ExternalInput").ap()
    # wq|wk|wv packed [C, 3*128]; wo [128, C]; gnp|gind packed [128, 34]
    wqkv_d = nc.dram_tensor("wqkv", [C, 6 * HD], F32, kind="ExternalInput").ap()
    wo_d = nc.dram_tensor("wo", [2 * HD, C], F32, kind="ExternalInput").ap()
    gg_d = nc.dram_tensor("gg", [128, 36], F32, kind="ExternalInput").ap()
    gbc_d = nc.dram_tensor("gbc", [16, C], F32, kind="ExternalInput").ap()
    y_d = nc.dram_tensor("y", [C, HW], mybir.dt.float16, kind="ExternalOutput").ap()

    with tile.TileContext(nc) as tc:
        with (
            tc.tile_pool(name="consts", bufs=1) as consts,
            tc.tile_pool(name="big", bufs=1) as big,
            tc.tile_pool(name="small", bufs=4) as small,
            tc.tile_pool(name="pt", bufs=8) as ptp,
        ):
            # ---- x first: it gates GroupNorm; spread enqueues across engines ----
            x_sb = [big.tile([128, HW], F32, tag=f"x{ct}", name=f"x{ct}") for ct in range(2)]
            # x0 first; x1 queued behind the small weight DMAs so the DMA engines
            # finish x0 early and tile-0 GroupNorm can start ~6us in
            nc.sync.dma_start(x_sb[0][:], x_d[0:128, :])
            gg_sb = consts.tile([128, 36], F32, tag="gg", name="gg")
            nc.gpsimd.dma_start(gg_sb[:], gg_d[:])
            gnp_sb = [gg_sb[:, 0:2], gg_sb[:, 2:4]]  # per-tile (weight, bias)
            gind_sb = gg_sb[:, 4:36]
            gbc_sb = consts.tile([16, C], F32, tag="gbc", name="gbc")
            nc.gpsimd.dma_start(gbc_sb[:], gbc_d[:])

            # preload the Exp ACT table set during the x DMA (the only set used)
            warm = small.tile([1, 2], F32, tag="warm", name="warm")
            nc.scalar.activation(warm[:], gg_sb[0:1, 0:2], AF.Exp, scale=0.0)

            # ---- weights (one DMA; casts on GpSimd to keep DVE free for GN) ----
            wqkvf = consts.tile([128, 2 * 6 * HD], F32, tag="wqkvf", name="wqkvf")
            wqf3 = wqkvf[:].rearrange("p (kc w) -> p kc w", kc=2)
            nc.sync.dma_start(
                wqf3, wqkv_d[:].rearrange("(kc p) w -> p kc w", kc=2)
            )
            w_sb = {}
            for wi, name in enumerate(("wq", "wk", "wv")):
                for kc in range(2):
                    w_sb[name, kc] = consts.tile(
                        [128, 2 * HD], mm_dt, tag=f"{name}{kc}", name=f"{name}{kc}"
                    )

            def emit_weight_casts(names):
                for wi, name in enumerate(("wq", "wk", "wv")):
                    if name not in names:
                        continue
                    for kc in range(2):
                        nc.vector.tensor_copy(
                            w_sb[name, kc][:], wqf3[:, kc, wi * 128 : (wi + 1) * 128]
                        )
            wof = consts.tile([128, C], F32, tag="wof", name="wof")
            nc.sync.dma_start(wof[:], wo_d[:])
            nc.scalar.dma_start(x_sb[1][:], x_d[128:256, :])
            wo_sb = consts.tile([128, C], mm_dt, tag="wo", name="wo")

            xn_sb = [big.tile([128, HW], mm_dt, tag=f"xn{ct}", name=f"xn{ct}") for ct in range(2)]

            # ---- GroupNorm ----
            stats = small.tile([128, 4], F32, tag="stats", name="stats")
            for ct in range(2):
                nc.vector.reduce_sum(stats[:, 2 * ct : 2 * ct + 1], x_sb[ct][:], axis=AX)
                nc.vector.scalar_tensor_tensor(
                    xn_sb[ct][:], x_sb[ct][:], 1.0, x_sb[ct][:],
                    op0=OP.mult, op1=OP.mult,
                    accum_out=stats[:, 2 * ct + 1 : 2 * ct + 2],
                )
            with tc.tile_pool(name="ps_gn", bufs=2, space=bass.MemorySpace.PSUM) as ps_gn:
                g_ps = ps_gn.tile([16, 2], F32, tag="g", name="g")
                nc.tensor.matmul(g_ps[:], gind_sb[:, 0:16], stats[:, 0:2], start=True, stop=False)
                nc.tensor.matmul(g_ps[:], gind_sb[:, 16:32], stats[:, 2:4], start=False, stop=True)
                mall = small.tile([16, 2], F32, tag="mall", name="mall")
                nc.vector.tensor_scalar_mul(mall[:], g_ps[:], 1.0 / (GC * HW))
                msq = small.tile([16, 1], F32, tag="msq", name="msq")
                nc.vector.tensor_tensor(msq[:], mall[:, 0:1], mall[:, 0:1], op=OP.mult)
                ve = small.tile([16, 1], F32, tag="ve", name="ve")
                nc.vector.tensor_tensor(ve[:], mall[:, 1:2], msq[:], op=OP.subtract)
                ve2 = small.tile([16, 1], F32, tag="ve2", name="ve2")
                nc.vector.tensor_scalar_add(ve2[:], ve[:], EPS)
                # rstd via Quake rsqrt bit-trick + 2 Newton steps, all on DVE
                sh = small.tile([16, 1], mybir.dt.uint32, tag="sh", name="sh")
                nc.vector.tensor_scalar(
                    sh[:], ve2[:].bitcast(mybir.dt.uint32), 1, None,
                    op0=OP.logical_shift_right,
                )
                nt = small.tile([16, 1], mybir.dt.uint32, tag="nt", name="nt")
                nc.vector.tensor_scalar(
                    nt[:], sh[:], 0xFFFFFFFF, None, op0=OP.bitwise_xor
                )
                r0u = small.tile([16, 1], mybir.dt.uint32, tag="r0u", name="r0u")
                nc.vector.tensor_scalar(
                    r0u[:], nt[:], 0xFFFFFFFF - 0x5F3759DF, None, op0=OP.subtract
                )
                r = r0u[:].bitcast(F32)
                gvals = small.tile([16, 2], F32, tag="gvals", name="gvals")
                nc.vector.tensor_copy(gvals[:, 0:1], mall[:, 0:1])
                for it in range(2):
                    rr = small.tile([16, 1], F32, tag=f"rr{it}", name=f"rr{it}")
                    nc.vector.tensor_tensor(rr[:], r, r, op=OP.mult)
                    hv = small.tile([16, 1], F32, tag=f"hv{it}", name=f"hv{it}")
                    nc.vector.tensor_tensor(hv[:], ve2[:], rr[:], op=OP.mult)
                    h2 = small.tile([16, 1], F32, tag=f"h2{it}", name=f"h2{it}")
                    nc.vector.tensor_scalar(
                        h2[:], hv[:], -0.5, 1.5, op0=OP.mult, op1=OP.add
                    )
                    if it == 0:
                        rn = small.tile([16, 1], F32, tag="rn", name="rn")
                        nc.vector.tensor_tensor(rn[:], r, h2[:], op=OP.mult)
                        r = rn[:]
                    else:
                        nc.vector.tensor_tensor(gvals[:, 1:2], r, h2[:], op=OP.mult)
                for ct in range(2):
                    cv = ps_gn.tile([128, 2], F32, tag="cv", name="cv")
                    nc.tensor.matmul(
                        cv[:], gbc_sb[:, ct * 128 : (ct + 1) * 128], gvals[:],
                        start=True, stop=True,
                    )
                    scale_t = small.tile([128, 1], F32, tag="scale", name="scale")
                    nc.vector.tensor_tensor(scale_t[:], gnp_sb[ct][:, 0:1], cv[:, 1:2], op=OP.mult)
                    tb = small.tile([128, 1], F32, tag="tb", name="tb")
                    nc.vector.tensor_tensor(tb[:], cv[:, 0:1], scale_t[:], op=OP.mult)
                    bias_t = small.tile([128, 1], F32, tag="bias", name="bias")
                    nc.vector.tensor_tensor(bias_t[:], gnp_sb[ct][:, 1:2], tb[:], op=OP.subtract)
                    nc.vector.tensor_scalar(
                        xn_sb[ct][:], x_sb[ct][:], scale_t[:], bias_t[:],
                        op0=OP.mult, op1=OP.add,
                    )
                emit_weight_casts(("wq", "wk", "wv"))
                nc.vector.tensor_copy(wo_sb[:], wof[:])

            # ---- QKV projections ----
            q_sb = big.tile([128, HW], mm_dt, tag="q", name="q")
            k_sb = big.tile([128, HW], mm_dt, tag="k", name="k")
            vt_sb = []
            for h in range(2):
                t = big.tile([128, NJ * (HD + 1)], mm_dt, tag=f"vt{h}", name=f"vt{h}")
                t3 = t[:].rearrange("p (j c) -> p j c", c=HD + 1)
                # fill the per-chunk "ones" column (denominator accumulator rows)
                nc.vector.tensor_scalar(
                    t3[:, :, HD : HD + 1], x_sb[0][:, 0:NJ], 0.0, 1.0,
                    op0=OP.mult, op1=OP.add,
                )
                vt_sb.append(t)

            with tc.tile_pool(name="ps_qkv", bufs=2, space=bass.MemorySpace.PSUM) as ps_qkv:
                # v^T first: its DVE copies then overlap the q/k matmuls, and
                # q/k (which attention needs first) are ready right at the boundary
                for half in range(2):
                    vps = ps_qkv.tile([128, HALF], F32, tag="qkv", name="qkv")
                    for j9 in range(9):
                        jc = half * 9 + j9
                        for kc in range(2):
                            nc.tensor.matmul(
                                vps[:, j9 * 128 : (j9 + 1) * 128],
                                xn_sb[kc][:, jc * JC : (jc + 1) * JC],
                                w_sb["wv", kc][:],
                                start=(kc == 0), stop=(kc == 1),
                            )
                    vps3 = vps[:].rearrange("p (j c) -> p j c", c=128)
                    for h in range(2):
                        dst3 = vt_sb[h][:].rearrange("p (j c) -> p j c", c=HD + 1)
                        nc.vector.tensor_copy(
                            dst3[:, half * 9 : (half + 1) * 9, 0:HD],
                            vps3[:, :, h * HD : (h + 1) * HD],
                        )
                for dst, wname in ((q_sb, "wq"), (k_sb, "wk")):
                    for half in range(2):
                        ps = ps_qkv.tile([128, HALF], F32, tag="qkv", name="qkv")
                        for kc in range(2):
                            for n0, n1 in _nchunks(HALF):
                                nc.tensor.matmul(
                                    ps[:, n0:n1],
                                    w_sb[wname, kc][:],
                                    xn_sb[kc][:, half * HALF + n0 : half * HALF + n1],
                                    start=(kc == 0), stop=(kc == 1),
                                )
                        # ScalarE is idle during the QKV phase; keep DVE free
                        nc.scalar.copy(dst[:, half * HALF : (half + 1) * HALF], ps[:])

            # ---- attention ----
            headout = big.tile([128, HW], mm_dt, tag="headout", name="headout")
            with tc.tile_pool(name="ps_att", bufs=1, space=bass.MemorySpace.PSUM) as ps_att:

                def emit_s(b, jc):
                    i0, i1 = IBLKS[b]
                    blk = i1 - i0
                    st = ps_att.tile([128, 2 * SALIGN], F32, tag="s", name="s", bufs=3)
                    for h in range(2):
                        lhsT = k_sb[h * HD : (h + 1) * HD, jc * JC : (jc + 1) * JC]
                        for n0, n1 in _nchunks(blk, 512):
                            nc.tensor.matmul(
                                st[:, h * SALIGN + n0 : h * SALIGN + n1],
                                lhsT,
                                q_sb[h * HD : (h + 1) * HD, i0 + n0 : i0 + n1],
                                start=True, stop=True,
                            )
                    pt = ptp.tile([128, 2 * blk], mm_dt, tag="pt", name="pt")
                    if blk == SALIGN:
                        stv, ptw = st[:], pt[:]
                    else:
                        stv = st[:].rearrange("p (h x) -> p h x", h=2)[:, :, 0:blk]
                        ptw = pt[:].rearrange("p (h x) -> p h x", h=2)
                    if jc in DVE_JC:
                        ptv = ptw.bitcast(U16)
                        nc.vector.tensor_scalar(
                            ptv, stv, A_SCH, B_SCH, op0=OP.mult, op1=OP.add
                        )
                    else:
                        nc.scalar.activation(ptw, stv, AF.Exp, scale=1.0 / 16.0)
                    return pt

                def emit_pv(b, jc, pt, u):
                    blk = IBLKS[b][1] - IBLKS[b][0]
                    for h in range(2):
                        lhsT = vt_sb[h][:, jc * (HD + 1) : (jc + 1) * (HD + 1)]
                        for n0, n1 in _nchunks(blk, 512):
                            nc.tensor.matmul(
                                u[h][:, n0:n1],
                                lhsT,
                                pt[:, h * blk + n0 : h * blk + n1],
                                start=(jc == 0), stop=(jc == NJ - 1),
                            )

                def emit_norm(u, b):
                    i0, i1 = IBLKS[b]
                    blk = i1 - i0
                    for h in range(2):
                        # stage the whole u tile to SBUF with ONE ScalarE copy (its
                        # cost is free-dim-bound, partitions are free) -- u's PSUM
                        # bank is then released for the next block's PV right away.
                        # reciprocal + multiply run off the copy; GpSimd does only
                        # partition_broadcast (one ucode library -- mixing op types
                        # thrashes LOAD_LIB at ~6us each)
                        dn = small.tile([1, blk], F32, tag=f"dn{h}", name=f"dn{h}")
                        nc.scalar.copy(dn[:], u[h][HD : HD + 1, :])
                        rc = small.tile([1, blk], F32, tag=f"rc{h}", name=f"rc{h}")
                        nc.vector.reciprocal_approx_fast(rc[:], dn[:])
                        rb = small.tile([HD, blk], F32, tag=f"rb{h}", name=f"rb{h}")
                        nc.gpsimd.partition_broadcast(rb[:], rc[:])
                        nc.vector.tensor_tensor(
                            headout[h * HD : (h + 1) * HD, i0:i1],
                            u[h][0:HD, :], rb[:], op=OP.mult,
                        )

                seq = [(b, jc) for b in range(NB) for jc in range(NJ)]
                pts = {}
                emitted = 0
                state = {"u": None}
                pvq = []

                def flush_pv():
                    bb, jj, pt = pvq.pop(0)
                    if jj == 0:
                        blk = IBLKS[bb][1] - IBLKS[bb][0]
                        state["u"] = [
                            ps_att.tile([HD + 1, blk], F32, tag=f"u{h}", name=f"u{h}", bufs=1)
                            for h in range(2)
                        ]
                    emit_pv(bb, jj, pt, state["u"])

                for i, (b, jc) in enumerate(seq):
                    while emitted <= min(i + LOOKAHEAD, len(seq) - 1):
                        pts[emitted] = emit_s(*seq[emitted])
                        emitted += 1
                    pvq.append((b, jc, pts.pop(i)))
                    # lag PV emission by 2 slots: a PV blocked on exp (or, at a
                    # block start, on the previous block's normalize reading u)
                    # then sits behind 2 extra S's in the in-order PE queue, so
                    # the S->exp stream keeps both exp engines fed
                    while len(pvq) > 2:
                        flush_pv()
                    if jc == NJ - 1:
                        while pvq:
                            flush_pv()
                        emit_norm(state["u"], b)

            # ---- output projection (partial: this core's head pair) ----
            with tc.tile_pool(name="ps_out", bufs=2, space=bass.MemorySpace.PSUM) as ps_out:
                ci = 0
                for mt in range(2):
                    for n0, n1 in _nchunks(HW):
                        yp = ps_out.tile([128, 512], F32, tag="yp", name="yp")
                        nc.tensor.matmul(
                            yp[:, 0 : n1 - n0],
                            wo_sb[:, mt * 128 : (mt + 1) * 128],
                            headout[:, n0:n1],
                            start=True, stop=True,
                        )
                        yo = small.tile([128, 512], mybir.dt.float16, tag="yo", name="yo")
                        # alternate the drain copies across the two idle engines
                        if ci % 2 == 0:
                            nc.vector.tensor_copy(yo[:, 0 : n1 - n0], yp[:, 0 : n1 - n0])
                        else:
                            nc.scalar.copy(yo[:, 0 : n1 - n0], yp[:, 0 : n1 - n0])
                        ci += 1
                        nc.sync.dma_start(
                            y_d[mt * 128 : (mt + 1) * 128, n0:n1], yo[:, 0 : n1 - n0]
                        )

    nc.compile()
    return nc


def _consts():
    # gind[:, 0:16]: tile-0 channel -> group one-hot; [:, 16:32]: tile-1 channel -> group
    gind = np.zeros((128, 32), np.float32)
    for c in range(128):
        gind[c, c // GC] = 1.0
        gind[c, 16 + 8 + c // GC] = 1.0
    gbc = np.zeros((16, C), np.float32)
    for c in range(C):
        gbc[c // GC, c] = 1.0
    return gind, gbc


def make_in_maps(x, gn_weight, gn_bias, qkv_w, out_w, out_b):
    x = np.asarray(x, np.float32)
    qkv_w = np.asarray(qkv_w, np.float32)
    out_w = np.asarray(out_w, np.float32)
    gn_weight = np.asarray(gn_weight, np.float32)
    gn_bias = np.asarray(gn_bias, np.float32)
    xr = np.ascontiguousarray(x.reshape(B, C, HW))
    gind, gbc = _consts()
    # gg cols: 0:2 gnp tile0 (weight,bias), 2:4 gnp tile1, 4:36 gind one-hots
    in_maps = []
    for core in range(NCORES):
        b, hp = divmod(core, 2)
        heads = (2 * hp, 2 * hp + 1)
        qs = np.concatenate([qkv_w[n * 192 : n * 192 + 64] for n in heads], 0)
        ks = np.concatenate([qkv_w[n * 192 + 64 : n * 192 + 128] for n in heads], 0)
        vs = np.concatenate([qkv_w[n * 192 + 128 : n * 192 + 192] for n in heads], 0)
        wqkv = np.concatenate([qs.T, ks.T, vs.T], axis=1)  # [C, 384]
        gg = np.zeros((128, 36), np.float32)
        gg[:, 0] = gn_weight[0:128]
        gg[:, 1] = gn_bias[0:128]
        gg[:, 2] = gn_weight[128:256]
        gg[:, 3] = gn_bias[128:256]
        gg[:, 4:36] = gind
        in_maps.append({
            "x": xr[b],
            "wqkv": np.ascontiguousarray(wqkv),
            "wo": np.ascontiguousarray(out_w[:, hp * 128 : (hp + 1) * 128].T),
            "gg": gg,
            "gbc": gbc,
        })
    return in_maps


def assemble(results, x, out_b):
    """Sum core-pair partial outputs and add the residual + bias on the host."""
    x = np.asarray(x, np.float32)
    out_b = np.asarray(out_b, np.float32)
    xr = x.reshape(B, C, HW)
    y = np.empty((B, C, HW), np.float32)
    for b in range(B):
        y[b] = (results[2 * b]["y"].astype(np.float32)
                + results[2 * b + 1]["y"].astype(np.float32)
                + xr[b] + out_b[:, None])
    return y.reshape(B, C, H, W)


_NC_CACHE = {}


def get_nc(mm_dt=BF16):
    key = str(mm_dt)
    if key not in _NC_CACHE:
        _NC_CACHE[key] = _build(mm_dt)
    return _NC_CACHE[key]


def kernel(x, gn_weight, gn_bias, qkv_w, out_w, out_b):
    nc = get_nc(BF16)
    in_maps = make_in_maps(x, gn_weight, gn_bias, qkv_w, out_w, out_b)
    res = bass_utils.run_bass_kernel_spmd(nc, in_maps, core_ids=list(range(NCORES)))
    return assemble(res.results, x, out_b)
